# revision 18
# baseline (speedup 1.0000x reference)
"""AttnBlock (GroupNorm + 4096-token single-head attention + residual) on 8 trn2 cores.

Sharding: 2 cores per batch sample. Each core computes GroupNorm + K/V for the
full sample (duplicated within the pair) and attention for half the queries
(2048 of 4096). A single SPMD program serves both halves: the host rotates the
sample's spatial columns so each core's query half always sits at columns
0..2047 (attention is permutation-invariant over keys; GroupNorm stats are
permutation-invariant over spatial positions).

Compute layout (per core):
  h = GN(x) [C=512, N=4096] bf16
  k = wk @ h [C, N] bf16;  q = wq @ h[:, :2048] [C, 2048] bf16
  vT = h^T @ wv^T [N, C] bf16  (natural matmul output layout, no transposes)
  scores^T[nk, q] = k^T q  (softmax denominator and PV both want nk on
  partitions; softmax itself needs no max-subtraction: scores ~ N(0,1))
  e = exp(scores * C^-0.5) bf16
  out_T[c, q] += v^T[nk,c].T @ e ; denom[1, q] += ones[nk,1].T @ e
  attn_out = out_T * (1/denom broadcast via rank-1 matmul)
  out = wo @ attn_out + bo + residual(x streamed back from DRAM)

SBUF lifetimes are phase-scoped: x (+weight staging) dies after GN/transposes,
then the k/q/vT pool opens in the freed space.
"""

import sys

for _p in ("/opt/trn_rl_repo", "/root/.axon_site/_ro/trn_rl_repo"):
    if _p not in sys.path:
        sys.path.append(_p)

import numpy as np

C = 512
N = 4096
NQ = 2048
P = 128
CT = C // P  # 4 c-tiles
NKB = N // P  # 32 nk blocks
QCH = NQ // 512  # 4 q chunks of 512
EPS = 1e-5
SCALE = float(C) ** -0.5

_cache = {}


def _build():
    import concourse.bacc as bacc
    import concourse.bass as bass
    import concourse.mybir as mybir
    import concourse.tile as tile
    from concourse.masks import make_identity

    f32 = mybir.dt.float32
    bf16 = mybir.dt.bfloat16
    AF = mybir.ActivationFunctionType
    ALU = mybir.AluOpType
    AX = mybir.AxisListType

    nc = bacc.Bacc("TRN2", target_bir_lowering=False, debug=False, num_devices=8)

    x_d = nc.dram_tensor("x", [C, N], f32, kind="ExternalInput")
    w_d = {
        nm: nc.dram_tensor(nm, [C, C], f32, kind="ExternalInput")
        for nm in ("wq", "wk", "wv", "wo")
    }
    vec_d = {
        nm: nc.dram_tensor(nm, [C], f32, kind="ExternalInput")
        for nm in ("bq", "bk", "bv", "bo", "gnw", "gnb")
    }
    out_d = nc.dram_tensor("out", [C, NQ], f32, kind="ExternalOutput")

    x_t = x_d.ap().rearrange("(t p) n -> t p n", p=P)
    out_t = out_d.ap().rearrange("(t p) n -> t p n", p=P)

    with tile.TileContext(nc) as tc:
        with (
            tc.tile_pool(name="const", bufs=1) as const,
            tc.tile_pool(name="work", bufs=3) as work,
            tc.tile_pool(name="wtp", bufs=1) as wtp,
            tc.tile_pool(name="hp", bufs=1) as hp,
            tc.tile_pool(name="ps_s", bufs=2, space="PSUM") as ps_s,
            tc.tile_pool(name="ps_o", bufs=4, space="PSUM") as ps_o,
            tc.tile_pool(name="ps_d", bufs=2, space="PSUM") as ps_d,
        ):
            # ---- constants ----
            ident = const.tile([P, P], f32)
            make_identity(nc, ident)
            ones_col = const.tile([P, 1], bf16)
            nc.vector.memset(ones_col, 1.0)
            ones_row = const.tile([1, P], f32)
            nc.vector.memset(ones_row, 1.0)
            eps_t = const.tile([P, 1], f32)
            nc.vector.memset(eps_t, EPS)

            # element-gather DMAs go on the gpsimd queue so the weight DMAs
            # lead the sync queue (PE transposes unblock first)
            cols = {}
            for nm in ("bq", "bk", "bo", "gnw", "gnb"):
                cols[nm] = const.tile([P, CT], f32, tag=f"col_{nm}", name=f"col_{nm}")
                nc.gpsimd.dma_start(
                    cols[nm], vec_d[nm].ap().rearrange("(t p) -> p t", p=P)
                )
            bv_row = const.tile([1, C], f32)
            nc.gpsimd.dma_start(
                bv_row, vec_d["bv"].ap().rearrange("(a c) -> a c", a=1)
            )

            # bv broadcast across partitions (for vT whose bias is on free dim)
            psb0 = ps_d.tile([P, 512], f32, tag="d")
            nc.tensor.matmul(psb0, lhsT=ones_row, rhs=bv_row, start=True, stop=True)
            bv_bcast = const.tile([P, C], f32)
            nc.vector.tensor_copy(bv_bcast, psb0)

            # wT tiles persist to the end (woT used in proj)
            wT = {
                nm: [
                    wtp.tile([P, C], bf16, tag=f"{nm}T{ci}", name=f"{nm}T{ci}")
                    for ci in range(CT)
                ]
                for nm in ("wq", "wk", "wv", "wo")
            }
            ht = [hp.tile([P, N], bf16, tag=f"h{t}", name=f"h{t}") for t in range(CT)]

            with tc.tile_pool(name="xp", bufs=1) as xp:
                # ---- load + transpose weights first (PE warms up on the
                # transposes while x streams in; fp32 has no DMA transpose) ----
                for nm in ("wk", "wq", "wv", "wo"):
                    wsb = xp.tile([P, CT, C], f32, tag="wstage", bufs=2)
                    nc.sync.dma_start(
                        wsb, w_d[nm].ap().rearrange("(t p) c -> p t c", p=P)
                    )
                    for t in range(CT):
                        for cs in range(CT):
                            pst = ps_o.tile([P, P], f32, tag="o", name="pst")
                            nc.tensor.transpose(
                                pst, wsb[:, t, cs * P : (cs + 1) * P], ident
                            )
                            nc.vector.tensor_copy(
                                out=wT[nm][cs][:, t * P : (t + 1) * P], in_=pst
                            )

                # ---- load x ----
                xt = []
                for t in range(CT):
                    xtile = xp.tile([P, N], f32, tag=f"x{t}")
                    nc.sync.dma_start(xtile, x_t[t])
                    xt.append(xtile)

                # ---- GroupNorm stats ----
                # per-channel mean/E[x^2] -> transpose -> group-sum (16
                # consecutive channels per group land in one row after
                # transpose) -> broadcast back -> per-channel scale/bias cols.
                # mvpack holds per-channel SUM (cols 0..3) and SUM of squares
                # (cols 4..7); the /(16*4096) happens at the group-sum scaling.
                mvpack = const.tile([P, 2 * CT], f32)
                for t in range(CT):
                    nc.vector.tensor_reduce(
                        out=mvpack[:, t : t + 1], in_=xt[t], axis=AX.X, op=ALU.add
                    )
                    junk = xp.tile([P, N], bf16, tag="junk", bufs=2)
                    nc.scalar.activation(
                        out=junk,
                        in_=xt[t],
                        func=AF.Square,
                        accum_out=mvpack[:, CT + t : CT + t + 1],
                    )

                pst1 = ps_o.tile([8, P], f32, tag="o", name="pst1")
                nc.tensor.transpose(pst1, mvpack, ident)
                statsT = const.tile([8, P], f32)
                nc.vector.tensor_copy(statsT, pst1)
                gsum = const.tile([8, 8], f32)
                nc.vector.tensor_reduce(
                    out=gsum,
                    in_=statsT.rearrange("p (g s) -> p g s", s=16),
                    axis=AX.X,
                    op=ALU.add,
                )
                nc.vector.tensor_scalar_mul(gsum, gsum, 1.0 / (16.0 * 4096.0))
                bcast16 = const.tile([8, P], f32)
                gsum_rep = bass.AP(
                    tensor=gsum.tensor,
                    offset=gsum.offset,
                    ap=list(gsum.ap) + [[0, 16]],
                )
                nc.vector.tensor_copy(
                    out=bcast16.rearrange("p (g s) -> p g s", s=16), in_=gsum_rep
                )
                pst2 = ps_o.tile([P, 8], f32, tag="o", name="pst2")
                nc.tensor.transpose(pst2, bcast16, ident[:8, :8])
                gcols = const.tile([P, 2 * CT], f32)
                nc.vector.tensor_copy(gcols, pst2)

                var_c = const.tile([P, CT], f32)
                nc.vector.tensor_mul(var_c, gcols[:, 0:CT], gcols[:, 0:CT])
                nc.vector.tensor_sub(var_c, gcols[:, CT : 2 * CT], var_c)
                rstd_c = const.tile([P, CT], f32)
                nc.scalar.activation(out=rstd_c, in_=var_c, func=AF.Sqrt, bias=eps_t)
                nc.vector.reciprocal(rstd_c, rstd_c)
                scale_c = const.tile([P, CT], f32)
                nc.vector.tensor_mul(scale_c, rstd_c, cols["gnw"])
                bias_c = const.tile([P, CT], f32)
                nc.vector.tensor_mul(bias_c, gcols[:, 0:CT], scale_c)
                nc.vector.tensor_sub(bias_c, cols["gnb"], bias_c)

                # ---- h = GN(x) in bf16 ----
                for t in range(CT):
                    nc.vector.tensor_scalar(
                        out=ht[t],
                        in0=xt[t],
                        scalar1=scale_c[:, t : t + 1],
                        scalar2=bias_c[:, t : t + 1],
                        op0=ALU.mult,
                        op1=ALU.add,
                    )
            # xp closed: x + weight staging space freed

            with tc.tile_pool(name="kqv", bufs=1) as kqv:
                kt = [
                    kqv.tile([P, N], bf16, tag=f"k{t}", name=f"k{t}")
                    for t in range(CT)
                ]
                qt = [
                    kqv.tile([P, NQ], bf16, tag=f"q{t}", name=f"q{t}")
                    for t in range(CT)
                ]
                # ---- k [C, N], q [C, NQ] ----
                for t in range(CT):
                    for nb in range(N // 512):
                        ps = ps_s.tile([P, 512], f32, tag="s")
                        for ci in range(CT):
                            nc.tensor.matmul(
                                ps,
                                lhsT=wT["wk"][ci][:, t * P : (t + 1) * P],
                                rhs=ht[ci][:, nb * 512 : (nb + 1) * 512],
                                start=(ci == 0),
                                stop=(ci == CT - 1),
                            )
                        nc.scalar.activation(
                            out=kt[t][:, nb * 512 : (nb + 1) * 512],
                            in_=ps,
                            func=AF.Identity,
                            bias=cols["bk"][:, t : t + 1],
                        )
                for t in range(CT):
                    for nb in range(NQ // 512):
                        ps = ps_s.tile([P, 512], f32, tag="s")
                        for ci in range(CT):
                            nc.tensor.matmul(
                                ps,
                                lhsT=wT["wq"][ci][:, t * P : (t + 1) * P],
                                rhs=ht[ci][:, nb * 512 : (nb + 1) * 512],
                                start=(ci == 0),
                                stop=(ci == CT - 1),
                            )
                        nc.scalar.activation(
                            out=qt[t][:, nb * 512 : (nb + 1) * 512],
                            in_=ps,
                            func=AF.Identity,
                            bias=cols["bq"][:, t : t + 1],
                        )

                # ---- vT [N, C] ----
                vt = []
                for nb in range(NKB):
                    ps = ps_o.tile([P, 512], f32, tag="o")
                    for ci in range(CT):
                        nc.tensor.matmul(
                            ps,
                            lhsT=ht[ci][:, nb * P : (nb + 1) * P],
                            rhs=wT["wv"][ci],
                            start=(ci == 0),
                            stop=(ci == CT - 1),
                        )
                    v = kqv.tile([P, C], bf16, tag=f"vt{nb}", name=f"vt{nb}")
                    nc.vector.tensor_add(out=v, in0=ps, in1=bv_bcast)
                    vt.append(v)

                # ---- attention ----
                for qc in range(QCH):
                    qs = qc * 512
                    pso = [
                        ps_o.tile([P, 512], f32, tag="o", name="pso")
                        for _ in range(CT)
                    ]
                    psd = ps_d.tile([1, 512], f32, tag="d")

                    def emit_scores(j):
                        pss = ps_s.tile([P, 512], f32, tag="s", name="pss")
                        for ci in range(CT):
                            nc.tensor.matmul(
                                pss,
                                lhsT=kt[ci][:, j * P : (j + 1) * P],
                                rhs=qt[ci][:, qs : qs + 512],
                                start=(ci == 0),
                                stop=(ci == CT - 1),
                            )
                        et = work.tile([P, 512], bf16, tag="exp", bufs=4, name="et")
                        nc.scalar.activation(out=et, in_=pss, func=AF.Exp, scale=SCALE)
                        return et

                    # 1-deep software pipeline: scores for j+1 are emitted
                    # before PV of j so the PE never waits on the Exp latency
                    et_cur = emit_scores(0)
                    for j in range(NKB):
                        et_next = emit_scores(j + 1) if j + 1 < NKB else None
                        for co in range(CT):
                            nc.tensor.matmul(
                                pso[co],
                                lhsT=vt[j][:, co * P : (co + 1) * P],
                                rhs=et_cur,
                                start=(j == 0),
                                stop=(j == NKB - 1),
                            )
                        nc.tensor.matmul(
                            psd,
                            lhsT=ones_col,
                            rhs=et_cur,
                            start=(j == 0),
                            stop=(j == NKB - 1),
                        )
                        et_cur = et_next
                    # Softmax division is deferred past the projection: the PV
                    # accumulators release after a plain bf16 cast, so the
                    # reciprocal/broadcast chain overlaps the proj matmuls
                    # instead of stalling the PE at the chunk boundary. The
                    # denominator row bounces through DRAM for the partition
                    # broadcast; the reciprocal runs full-width afterwards (a
                    # [1,512] single-partition reciprocal measures 3.3us on DVE).
                    aoT = []
                    for co in range(CT):
                        a = work.tile([P, 512], bf16, tag=f"ao{co}", bufs=1, name="ao")
                        nc.vector.tensor_copy(a, pso[co])
                        aoT.append(a)
                    dsb = work.tile([1, 512], f32, tag="dsb", bufs=2)
                    nc.vector.tensor_copy(dsb, psd)
                    psb = ps_d.tile([P, 512], f32, tag="d", name="psb")
                    nc.tensor.matmul(psb, lhsT=ones_row, rhs=dsb, start=True, stop=True)
                    rdb = work.tile([P, 512], f32, tag="rdb", bufs=2)
                    nc.vector.reciprocal(rdb, psb)
                    for co in range(CT):
                        xres = work.tile([P, 512], f32, tag="xres", bufs=4)
                        nc.sync.dma_start(xres, x_t[co][:, qs : qs + 512])
                        nc.vector.tensor_scalar_add(
                            out=xres, in0=xres, scalar1=cols["bo"][:, co : co + 1]
                        )
                        psp = ps_d.tile([P, 512], f32, tag="d", name="psp")
                        for ci in range(CT):
                            nc.tensor.matmul(
                                psp,
                                lhsT=wT["wo"][ci][:, co * P : (co + 1) * P],
                                rhs=aoT[ci],
                                start=(ci == 0),
                                stop=(ci == CT - 1),
                            )
                        osb = work.tile([P, 512], f32, tag="osb", bufs=3)
                        nc.vector.tensor_mul(osb, psp, rdb)
                        nc.vector.tensor_add(out=osb, in0=osb, in1=xres)
                        nc.sync.dma_start(out_t[co][:, qs : qs + 512], osb)

    nc.compile()
    return nc


def _get_nc():
    if "nc" not in _cache:
        _cache["nc"] = _build()
    return _cache["nc"]


def kernel(**inputs):
    from concourse.bass_utils import run_bass_kernel_spmd

    nc = _get_nc()

    x = np.ascontiguousarray(np.asarray(inputs["hidden_states"], dtype=np.float32))
    B = x.shape[0]
    xs = x.reshape(B, C, N)
    common = {
        "wq": np.ascontiguousarray(np.asarray(inputs["wq"], np.float32)),
        "wk": np.ascontiguousarray(np.asarray(inputs["wk"], np.float32)),
        "wv": np.ascontiguousarray(np.asarray(inputs["wv"], np.float32)),
        "wo": np.ascontiguousarray(np.asarray(inputs["wo"], np.float32)),
        "bq": np.ascontiguousarray(np.asarray(inputs["bq"], np.float32)),
        "bk": np.ascontiguousarray(np.asarray(inputs["bk"], np.float32)),
        "bv": np.ascontiguousarray(np.asarray(inputs["bv"], np.float32)),
        "bo": np.ascontiguousarray(np.asarray(inputs["bo"], np.float32)),
        "gnw": np.ascontiguousarray(np.asarray(inputs["gn_w"], np.float32)),
        "gnb": np.ascontiguousarray(np.asarray(inputs["gn_b"], np.float32)),
    }
    in_maps = []
    for core in range(8):
        s, half = core // 2, core % 2
        xc = xs[s] if half == 0 else np.ascontiguousarray(np.roll(xs[s], -NQ, axis=1))
        in_maps.append({"x": xc, **common})

    res = run_bass_kernel_spmd(nc, in_maps, list(range(8)))

    out = np.empty((B, C, N), np.float32)
    for core in range(8):
        s, half = core // 2, core % 2
        out[s][:, half * NQ : (half + 1) * NQ] = res.results[core]["out"]
    return out.reshape(B, C, 64, 64)
